# revision 17
# baseline (speedup 1.0000x reference)
"""KAN layer (histogram binning) Trainium2 kernel.

Math reformulation (exact for linear interpolation on a uniform grid):
  proj = clip(x @ P, +-0.99)                         [N, 3]
  out  = tanh(W @ CPf)  where W[n, (g,c)] = relu(1 - 2.5*|proj[n,c] - grid[g]|)
         and CPf[(g,c), :] = control_points[c, g, :] * component_weights[c]

Kernel pipeline per 512-token quarter (8 cores x 8192 tokens each):
  DMA x in (natural [tok, d]) -> PE transpose to [d, tok] -> U = Q^T.T @ X^T
  (one fused matmul, Q = P replicated over grid rows) -> clip (DVE) ->
  |u - g| (DVE fused add+abs_max with per-partition bias) -> relu affine (ACT)
  -> out = W.T @ CPf per 128-token chunk (back to [tok, o] layout) -> tanh
  (ACT) -> DMA out.

PE inputs are typed float32r end-to-end (same 32-bit storage as f32 --
numpy binds f32 arrays -- but the PE streams it 4x faster; the BIR
verifier requires f32r provenance from the producer, so the DRAM
tensors and intermediate tiles are typed f32r, not bitcast at the
instruction site).
"""

import os
from contextlib import ExitStack

import numpy as np

import concourse.bass as bass
import concourse.bacc as bacc
import concourse.tile as tile
from concourse import mybir
from concourse.bass_utils import run_bass_kernel_spmd

N_CORES = 8
TOK_TOTAL = 32 * 2048
D = 256
O = 256
G = 6
C = 3
R = G * C  # 18 interp weights per token
SUPER = 2048  # tokens per supertile
QUART = 512
CHUNK = 128

F32 = mybir.dt.float32
F32R = mybir.dt.float32r

# f32r-at-instruction-site knobs
MMU_F32R = os.environ.get("KAN_MMU_F32R", "1") == "1"
MM3_F32R = os.environ.get("KAN_MM3_F32R", "1") == "1"
TRANS_F32R = os.environ.get("KAN_TRANS_F32R", "1") == "1"
# token-to-partition layout: PJ gives each partition a 16 KiB contiguous
# HBM segment per supertile DMA (tokens p*16+j) instead of 16 strided 1 KiB
# segments (tokens j*128+p). Same permutation applied on input and output,
# so per-token math is unaffected.
LAYOUT_PJ = os.environ.get("KAN_LAYOUT_PJ", "1") == "1"


def _dt(use_f32r):
    return F32R if use_f32r else F32


def build_nc(tok_per_core: int, n_cores: int = N_CORES, n_rep: int = 1,
             mmu_f32r=None, mm3_f32r=None, trans_f32r=None, layout_pj=None):
    mmu_f32r = MMU_F32R if mmu_f32r is None else mmu_f32r
    mm3_f32r = MM3_F32R if mm3_f32r is None else mm3_f32r
    trans_f32r = TRANS_F32R if trans_f32r is None else trans_f32r
    layout_pj = LAYOUT_PJ if layout_pj is None else layout_pj
    n_super = tok_per_core // SUPER
    assert tok_per_core % SUPER == 0

    nc = bacc.Bacc(
        "TRN2", target_bir_lowering=False, debug=False, num_devices=n_cores
    )
    trans_dt = _dt(trans_f32r)
    mmu_dt = _dt(mmu_f32r)
    mm3_dt = _dt(mm3_f32r)
    x_d = nc.dram_tensor("x", [tok_per_core, D], trans_dt, kind="ExternalInput").ap()
    q_d = nc.dram_tensor("qmat", [128, 2 * R], mmu_dt, kind="ExternalInput").ap()
    cp_d = nc.dram_tensor("cpb", [32, O], mm3_dt, kind="ExternalInput").ap()
    b_d = nc.dram_tensor("biasp", [32, 1], F32, kind="ExternalInput").ap()
    id_d = nc.dram_tensor("ident", [128, 128], trans_dt, kind="ExternalInput").ap()
    out_d = nc.dram_tensor("out", [tok_per_core, D], F32, kind="ExternalOutput").ap()

    if layout_pj:
        x_v = x_d.rearrange("(s p j) d -> s p j d", p=128, j=SUPER // CHUNK)
        o_v = out_d.rearrange("(s p j) d -> s p j d", p=128, j=SUPER // CHUNK)
    else:
        x_v = x_d.rearrange("(s j p) d -> s p j d", p=128, j=SUPER // CHUNK)
        o_v = out_d.rearrange("(s j p) d -> s p j d", p=128, j=SUPER // CHUNK)

    with tile.TileContext(nc) as tc, ExitStack() as ctx:
        const_p = ctx.enter_context(tc.tile_pool(name="const", bufs=1))
        xn_p = ctx.enter_context(tc.tile_pool(name="xn", bufs=2))
        xtps_p = ctx.enter_context(tc.tile_pool(name="xtps", bufs=3, space="PSUM"))
        xtsb_p = ctx.enter_context(tc.tile_pool(name="xtsb", bufs=3))
        ups_p = ctx.enter_context(tc.tile_pool(name="ups", bufs=2, space="PSUM"))
        w_p = ctx.enter_context(tc.tile_pool(name="w", bufs=2))
        ops_p = ctx.enter_context(tc.tile_pool(name="ops", bufs=3, space="PSUM"))
        osb_p = ctx.enter_context(tc.tile_pool(name="osb", bufs=2))

        ident = const_p.tile([128, 128], trans_dt)
        nc.sync.dma_start(ident[:], id_d)
        qmat = const_p.tile([128, 2 * R], mmu_dt)
        nc.sync.dma_start(qmat[:], q_d)
        cpb = const_p.tile([32, O], mm3_dt)
        nc.sync.dma_start(cpb[:], cp_d)
        biasp = const_p.tile([32, 1], F32)
        nc.sync.dma_start(biasp[:], b_d)

        for rep in range(n_rep):
          for s in range(n_super):
            xn = xn_p.tile([128, SUPER // CHUNK * D], trans_dt)
            nc.sync.dma_start(
                xn.rearrange("p (j d) -> p j d", j=SUPER // CHUNK), x_v[s]
            )
            out_sb = osb_p.tile([128, SUPER // CHUNK * O], F32)
            for q in range(SUPER // QUART):
                xt_sb = []
                for h in range(2):
                    xt_ps = xtps_p.tile([128, QUART], trans_dt, tag="xtps")
                    for c in range(QUART // CHUNK):
                        j = (SUPER // QUART) * q + c
                        nc.tensor.transpose(
                            xt_ps[:, CHUNK * c : CHUNK * (c + 1)],
                            xn[:, j * D + 128 * h : j * D + 128 * (h + 1)],
                            ident[:],
                        )
                    sb = xtsb_p.tile([128, QUART], mmu_dt, tag="xtsb")
                    nc.vector.tensor_copy(sb[:], xt_ps[:])
                    xt_sb.append(sb)
                u_ps = ups_p.tile([128, QUART], F32, tag="ups")
                for h in range(2):
                    nc.tensor.matmul(
                        u_ps[0:R, :],
                        qmat[:, R * h : R * (h + 1)],
                        xt_sb[h][:],
                        start=(h == 0),
                        stop=(h == 1),
                    )
                u_sb = w_p.tile([32, QUART], F32, tag="usb")
                nc.vector.tensor_scalar(
                    u_sb[0:R, :],
                    u_ps[0:R, :],
                    0.99,
                    -0.99,
                    op0=mybir.AluOpType.min,
                    op1=mybir.AluOpType.max,
                )
                a_sb = w_p.tile([32, QUART], F32, tag="asb")
                nc.scalar.activation(
                    a_sb[0:R, :],
                    u_sb[0:R, :],
                    mybir.ActivationFunctionType.Abs,
                    bias=biasp[0:R, :],
                    scale=1.0,
                )
                w_sb = w_p.tile([32, QUART], mm3_dt, tag="wsb")
                nc.scalar.activation(
                    w_sb[0:R, :],
                    a_sb[0:R, :],
                    mybir.ActivationFunctionType.Relu,
                    bias=1.0,
                    scale=-2.5,
                )
                for cp_i in range(2):
                    out_ps = ops_p.tile([128, 2 * O], F32, tag="ops")
                    for ce in range(2):
                        c = 2 * cp_i + ce
                        nc.tensor.matmul(
                            out_ps[:, O * ce : O * (ce + 1)],
                            w_sb[0:R, CHUNK * c : CHUNK * (c + 1)],
                            cpb[0:R, :],
                            start=True,
                            stop=True,
                        )
                    off = ((SUPER // QUART) * q + 2 * cp_i) * O
                    nc.scalar.activation(
                        out_sb[:, off : off + 2 * O],
                        out_ps[:],
                        mybir.ActivationFunctionType.Tanh,
                    )
            # output DMA rides the Activation-engine DGE queue so it never
            # serializes behind the next supertile's input DMA (SP queue)
            nc.scalar.dma_start(
                o_v[s], out_sb.rearrange("p (j d) -> p j d", j=SUPER // CHUNK)
            )

    nc.compile()
    return nc


def make_consts(projections: np.ndarray, control_points: np.ndarray,
                component_weights: np.ndarray):
    grid = np.linspace(-1.0, 1.0, G).astype(np.float32)
    qmat = np.zeros((128, 2 * R), np.float32)
    for h in range(2):
        for r in range(R):
            qmat[:, h * R + r] = projections[h * 128 : (h + 1) * 128, r % C]
    cpb = np.zeros((32, O), np.float32)
    biasp = np.zeros((32, 1), np.float32)
    for r in range(R):
        g, c = r // C, r % C
        cpb[r] = control_points[c, g] * component_weights[c]
        biasp[r, 0] = -grid[g]
    ident = np.eye(128, dtype=np.float32)
    return qmat, cpb, biasp, ident


_NC_CACHE = {}


def kernel(x, projections, control_points, component_weights, _trace=False):
    x = np.asarray(x)
    B, S, _ = x.shape
    tok = B * S
    tok_per_core = tok // N_CORES
    key = tok_per_core
    if key not in _NC_CACHE:
        _NC_CACHE[key] = build_nc(tok_per_core)
    nc = _NC_CACHE[key]

    qmat, cpb, biasp, ident = make_consts(
        np.asarray(projections), np.asarray(control_points),
        np.asarray(component_weights)
    )
    flat = np.ascontiguousarray(x.reshape(tok, D))
    in_maps = []
    for i in range(N_CORES):
        in_maps.append(
            {
                "x": flat[i * tok_per_core : (i + 1) * tok_per_core],
                "qmat": qmat,
                "cpb": cpb,
                "biasp": biasp,
                "ident": ident,
            }
        )
    res = run_bass_kernel_spmd(nc, in_maps, list(range(N_CORES)), trace=_trace)
    out = np.concatenate([res.results[i]["out"] for i in range(N_CORES)], axis=0)
    ret = out.reshape(B, S, O).astype(x.dtype, copy=False)
    if _trace:
        return ret, res
    return ret
